# revision 45
# baseline (speedup 1.0000x reference)
"""Trainium2 Bass kernel for Deformable-DETR multi-scale deformable attention.

8 NeuronCores, data-parallel over batch (1 batch element per core, SPMD, no
collectives).

Per core:
  Phase 1: V = X @ W_v + b_v on PE (fp16 in, fp32 PSUM), stored to DRAM
    "pair tables": per (head, level), unit u = e*W + x holds rows (e-1, e)
    of column x as [2, 32ch] fp16 (128 B). A bilinear 2x2x32 patch is then
    2 consecutive units (256 B). For dma_gather (256B-aligned chunks) each
    table region also carries a parity copy B = A shifted by one unit
    (built with one contiguous DRAM->DRAM copy), so chunk(u) =
    (u>>1) + (u&1)*B_off is 256B-aligned for any parity of u.
  Phase 2 (per 128-query group): offset/attention projections on PE
    (weights pre-permuted host-side to level-major slot order),
    softmax + sampling locations + masked corner weights on DVE,
    chunk indices wrapped into dma_gather's int16 index layout via 8
    PE replication matmuls, 7 dma_gather instructions (one per level /
    level-0 head-pair), fp16 broadcast-multiply + add-tree combine on
    DVE, output projection on PE.

All potentially-junk table reads (x-wrap at row ends, out-of-range
samples) are zero-masked via the per-corner validity folded into the
bilinear weights; every reachable table byte is written, zero-filled, or
B-copied so junk stays finite.
"""

import sys

sys.path.insert(0, "/opt/trn_rl_repo")

import dataclasses
import math

import numpy as np

import concourse.bass as bass
import concourse.mybir as mybir
import concourse.tile as tile
from concourse import bacc
from concourse.bass_utils import run_bass_kernel_spmd
from concourse.masks import make_identity

# ---------------- problem constants (hardcoded) ----------------
SPATIAL = [(100, 150), (50, 75), (25, 38), (13, 19)]
TOTAL = sum(h * w for h, w in SPATIAL)  # 19947
BS, Q, D, NH, NL, NP = 8, 900, 256, 8, 4, 4
DH = D // NH  # 32
NQG = math.ceil(Q / 128)  # 8 query groups (7x128 + 4)
MAGIC = 12582912.0  # 1.5 * 2**23: float32 round-to-nearest-int trick

FP32 = mybir.dt.float32
FP16 = mybir.dt.float16
INT16 = mybir.dt.int16
ADD = mybir.AluOpType.add
SUB = mybir.AluOpType.subtract
MUL = mybir.AluOpType.mult
MAXOP = mybir.AluOpType.max
MINOP = mybir.AluOpType.min

LVL_START = []
_s = 0
for _h, _w in SPATIAL:
    LVL_START.append(_s)
    _s += _h * _w
LVL_NT = [math.ceil(h * w / 128) for h, w in SPATIAL]
LVL_T0 = [sum(LVL_NT[:i]) for i in range(NL)]
NT_TOT = sum(LVL_NT)  # 158

# Pair-table geometry, in 128B units (1 unit = [2 rows, 32 ch] fp16).
# Per (level, head): A region of R units.  Level tensor layouts (units):
#   level 0: 4 pair-groups of [A_h, A_h+1, B_h, B_h+1] = 4R each
#   levels 1-3: one group [A_0..A_7, B_0..B_7] = 16R
# B = A shifted by one unit so any-parity 2-unit patches are 256B-aligned
# chunks: chunk(u) = (u>>1) + (u&1)*CA where CA = B chunk offset.
R_L = []
for _li, (_h, _w) in enumerate(SPATIAL):
    r = _w + LVL_NT[_li] * 128
    r += r & 1  # even
    R_L.append(r)
UNITS_L = [16 * r for r in R_L]
# group size in units / chunks, B chunk offset, per level
GRP_UNITS = [4 * R_L[0]] + [16 * r for r in R_L[1:]]
GRP_CHUNKS = [u // 2 for u in GRP_UNITS]
B_CHUNK_OFF = [R_L[0]] + [4 * r for r in R_L[1:]]
NGRP = [4, 1, 1, 1]

# level-major slot order: s = (l*NH + h)*NP + p
def _slot(l, h, p):
    return (l * NH + h) * NP + p


def _head_base(li, h):
    """A-region base (units) of head h inside its group."""
    if li == 0:
        return (h % 2) * R_L[0]
    return h * R_L[li]


def _grp_of_head(li, h):
    return h // 2 if li == 0 else 0


def _np_consts():
    """cvec [128, 9, 128] f32 (row-replicated per-slot constants) and
    rep16 [16, 128] f32 (replication selector)."""
    c = np.zeros((9, 128), np.float32)
    for li, (H, W) in enumerate(SPATIAL):
        for h in range(NH):
            for p in range(NP):
                s = _slot(li, h, p)
                c[0, s] = W  # Wvec
                c[1, s] = H  # Hvec
                c[2, s] = W - 1
                c[3, s] = W - 2
                c[4, s] = H - 1
                c[5, s] = H - 2
                c[6, s] = _head_base(li, h) + W  # u = bconst + y0*W + x0
                c[7, s] = 2.0 * B_CHUNK_OFF[li]  # CA2 (applied to par/2)
                c[8, s] = GRP_CHUNKS[li] - 2  # chunk clamp hi
    cvec = np.broadcast_to(c[None, :, :], (128, 9, 128)).copy()
    # rep4[64a + k, v, p] = 1 iff k == v*16 + p%16  (k in [0,64), halves a)
    rep = np.zeros((128, 4, 128), np.float32)
    for a in range(2):
        for k in range(64):
            for v in range(4):
                if 0 <= k - v * 16 < 16:
                    for p in range(128):
                        if p % 16 == k - v * 16:
                            rep[64 * a + k, v, p] = 1.0
    return cvec, rep.reshape(128, 4 * 128)


def _np_permuted_weights(W_off, b_off, W_attn, b_attn):
    """Permute projection weight columns to level-major slot order and fold
    the -0.5 pixel-center shift into b_off."""
    perm = np.zeros(128, np.int64)
    for h in range(NH):
        for l in range(NL):
            for p in range(NP):
                perm[_slot(l, h, p)] = (h * NL + l) * NP + p
    w_off2 = np.ascontiguousarray(
        W_off.reshape(D, 128, 2)[:, perm, :].reshape(D, D), np.float32
    )
    b_off2 = np.ascontiguousarray(
        b_off.reshape(128, 2)[perm, :].reshape(D), np.float32
    ) - 0.5
    w_attn2 = np.ascontiguousarray(W_attn[:, perm], np.float32)
    b_attn2 = np.ascontiguousarray(b_attn[perm], np.float32)
    return w_off2, b_off2.astype(np.float32), w_attn2, b_attn2


def _bc(ap, dim, count):
    """Insert a broadcast (step-0) dim at position `dim` of an AP."""
    new = list(ap.ap)
    new.insert(dim, [0, count])
    return dataclasses.replace(ap, ap=new)


# xf16 staging layout: per-level, padded to 1024-row bands
LVL_NB = [math.ceil(nt * 128 / 1024) for nt in LVL_NT]
XB = [sum(LVL_NB[:i]) * 1024 for i in range(NL)]
XROWS = sum(LVL_NB) * 1024


def build(reps: int = 1, ablate: set | None = None):
    ablate = ablate or set()
    nc = bacc.Bacc("TRN2", target_bir_lowering=False, debug=False, num_devices=8)

    t_q = nc.dram_tensor("query", [Q, D], FP32, kind="ExternalInput")
    t_rp = nc.dram_tensor("reference_points", [Q, NL * 2], FP32, kind="ExternalInput")
    t_x = nc.dram_tensor("input_flatten", [TOTAL, D], FP32, kind="ExternalInput")
    t_woff = nc.dram_tensor("W_off", [D, D], FP32, kind="ExternalInput")
    t_boff = nc.dram_tensor("b_off", [D], FP32, kind="ExternalInput")
    t_watt = nc.dram_tensor("W_attn", [D, 128], FP32, kind="ExternalInput")
    t_batt = nc.dram_tensor("b_attn", [128], FP32, kind="ExternalInput")
    t_wv = nc.dram_tensor("W_v", [D, D], FP32, kind="ExternalInput")
    t_bv = nc.dram_tensor("b_v", [D], FP32, kind="ExternalInput")
    t_wo = nc.dram_tensor("W_o", [D, D], FP32, kind="ExternalInput")
    t_bo = nc.dram_tensor("b_o", [D], FP32, kind="ExternalInput")
    t_cvec = nc.dram_tensor("cvec", [128, 9 * 128], FP32, kind="ExternalInput")
    t_rep = nc.dram_tensor("rep16", [128, 4 * 128], FP32, kind="ExternalInput")
    t_out = nc.dram_tensor("out", [Q, D], FP32, kind="ExternalOutput")

    with tile.TileContext(nc) as tc:
        with (
            tc.tile_pool(name="dram", bufs=1, space="DRAM") as dpool,
            tc.tile_pool(name="const", bufs=1) as cpool,
            tc.tile_pool(name="psum", bufs=2, space="PSUM") as ppool,
        ):
            tabg = []
            for _tli in range(NL):
                _row = []
                for _g in range(NGRP[_tli]):
                    _tabtile = dpool.tile(
                        [GRP_UNITS[_tli], 64], FP16, tag=f"table{_tli}_{_g}"
                    )
                    _row.append(_tabtile)
                tabg.append(_row)
            zdram = dpool.tile([1024, 64], FP16, tag="zdram")

            # ---- constants / weights ----
            ident = cpool.tile([128, 128], FP32)
            make_identity(nc, ident[:])
            wv16 = cpool.tile([128, 2, D], FP16)
            wo16 = cpool.tile([128, 2, D], FP16)
            woff = cpool.tile([128, 2, D], FP32)
            watt = cpool.tile([128, 2, 128], FP32)
            for j in range(2):
                nc.gpsimd.dma_start(out=wv16[:, j, :], in_=t_wv[j * 128 : (j + 1) * 128, :])
                nc.gpsimd.dma_start(out=wo16[:, j, :], in_=t_wo[j * 128 : (j + 1) * 128, :])
                nc.sync.dma_start(out=woff[:, j, :], in_=t_woff[j * 128 : (j + 1) * 128, :])
                nc.sync.dma_start(out=watt[:, j, :], in_=t_watt[j * 128 : (j + 1) * 128, :])
            bv16 = cpool.tile([1, D], FP16)
            bo16 = cpool.tile([1, D], FP16)
            boff = cpool.tile([1, D], FP32)
            batt = cpool.tile([1, 128], FP32)
            nc.gpsimd.dma_start(out=bv16[:], in_=t_bv[None, :])
            nc.gpsimd.dma_start(out=bo16[:], in_=t_bo[None, :])
            nc.sync.dma_start(out=boff[:], in_=t_boff[None, :])
            nc.sync.dma_start(out=batt[:], in_=t_batt[None, :])
            ones32 = cpool.tile([1, 128], FP32)
            ones16 = cpool.tile([1, 128], FP16)
            nc.vector.memset(ones32[:], 1.0)
            nc.vector.memset(ones16[:], 1.0)
            cvec = cpool.tile([128, 9, 128], FP32)
            nc.sync.dma_start(
                out=cvec[:, :, :], in_=t_cvec[:, :].rearrange("p (k s) -> p k s", k=9)
            )
            rep4 = cpool.tile([128, 4, 128], FP32)
            nc.sync.dma_start(
                out=rep4[:, :, :], in_=t_rep[:, :].rearrange("p (v s) -> p v s", v=4)
            )

            def cv(k):
                return cvec[:, k, :]

            # ---- zero-fill helper (issued on the Pool queue, SWDGE) ----
            zt = cpool.tile([128, 1, 64], FP16)
            nc.vector.memset(zt[:, :, :], 0.0)
            ident16 = cpool.tile([128, 128], FP16)
            nc.vector.tensor_copy(ident16[:, :], ident[:, :])

            nc.gpsimd.dma_start(
                out=zdram[:, :].rearrange("(t p) c -> p t c", p=128),
                in_=_bc(zt[:, 0, :], 1, 8),
            )

            def zfill(tab, u0, n):
                while n > 0:
                    k = min(n, 1024)
                    nc.sync.dma_start(out=tab[u0 : u0 + k, :], in_=zdram[:k, :])
                    u0 += k
                    n -= k

            def zfill_level(li):
                # coarse contiguous slack ranges: head lead-ins + tails merged
                W = SPATIAL[li][1]
                nt = LVL_NT[li]
                for g in range(NGRP[li]):
                    nh_in_g = 2 if li == 0 else NH
                    zfill(tabg[li][g], 0, W)
                    for j in range(nh_in_g):
                        b = j * R_L[li]
                        end = b + R_L[li] + (W if j + 1 < nh_in_g else 0)
                        zfill(tabg[li][g], b + nt * 128, end - (b + nt * 128))
                    zfill(tabg[li][g], GRP_UNITS[li] - 1, 1)

            for rep in range(reps):
              if rep:
                  tc.no_sync_barrier()
              with (
                  tc.tile_pool(name=f"p2w{rep}", bufs=2) as wp2,
                  tc.tile_pool(name=f"p2s{rep}", bufs=1) as sp2,
                  tc.tile_pool(name=f"p2b{rep}", bufs=4) as bigp,
                  tc.tile_pool(name=f"p2i{rep}", bufs=8) as ipool,
              ):
                  idx_t, cw_t, ogp_t, c32_t = {}, {}, {}, {}

                  # ===== Loop A body: per-qg addresses/weights ==============
                  def emit_A(qg):
                      q0 = qg * 128
                      nq = min(128, Q - q0)

                      qt32 = wp2.tile([128, D], FP32, tag="qt32")
                      nc.sync.dma_start(out=qt32[:nq, :], in_=t_q[q0 : q0 + nq, :])
                      qT = wp2.tile([128, 2, 128], FP32, tag="qT")
                      for j in range(2):
                          tp = ppool.tile([128, D], FP32, tag="mmout")
                          nc.tensor.transpose(
                              tp[:, :nq], qt32[:nq, j * 128 : (j + 1) * 128], ident[:nq, :nq]
                          )
                          nc.scalar.copy(out=qT[:, j, :nq], in_=tp[:, :nq])

                      offp = ppool.tile([128, D], FP32, tag="mmout")
                      nc.tensor.matmul(offp[:nq, :], qT[:, 0, :nq], woff[:, 0, :], start=True, stop=False)
                      nc.tensor.matmul(offp[:nq, :], qT[:, 1, :nq], woff[:, 1, :], start=False, stop=False)
                      nc.tensor.matmul(offp[:nq, :], ones32[:, :nq], boff[:], start=False, stop=True)
                      off = sp2.tile([128, D], FP32, tag="off")
                      nc.scalar.copy(out=off[:nq, :], in_=offp[:nq, :])

                      attp = ppool.tile([128, 128], FP32, tag="tpsum")
                      nc.tensor.matmul(attp[:nq, :], qT[:, 0, :nq], watt[:, 0, :], start=True, stop=False)
                      nc.tensor.matmul(attp[:nq, :], qT[:, 1, :nq], watt[:, 1, :], start=False, stop=False)
                      nc.tensor.matmul(attp[:nq, :], ones32[:, :nq], batt[:], start=False, stop=True)
                      att = sp2.tile([128, 128], FP32, tag="att")
                      nc.scalar.copy(out=att[:nq, :], in_=attp[:nq, :])

                      # softmax over (l,p)=16 per head; logits are tame
                      # (|x| <~ 6) so no max subtraction needed.
                      ex = sp2.tile([128, 128], FP32, tag="ex")
                      nc.scalar.activation(
                          ex[:nq, :], att[:nq, :], mybir.ActivationFunctionType.Exp,
                          bias=0.0, scale=1.0,
                      )
                      s16 = sp2.tile([128, 8], FP32, tag="s16")
                      nc.vector.tensor_reduce(
                          s16[:nq, :],
                          ex[:nq, :].rearrange("q (l h p) -> q h l p", l=NL, h=NH),
                          mybir.AxisListType.XY, ADD,
                      )
                      r16 = sp2.tile([128, 8], FP32, tag="r16")
                      nc.vector.reciprocal(r16[:nq, :], s16[:nq, :])
                      attn = sp2.tile([128, 128], FP32, tag="attn")
                      nc.vector.tensor_tensor(
                          attn[:nq, :].rearrange("q (l h p) -> q h l p", l=NL, h=NH),
                          ex[:nq, :].rearrange("q (l h p) -> q h l p", l=NL, h=NH),
                          _bc(_bc(r16[:nq, :], 2, NL), 3, NP),
                          MUL,
                      )

                      # sampling locations: xc = off_x + rp_x*W  (the -0.5 is
                      # folded into b_off host-side; off is already in pixels)
                      rxy = sp2.tile([128, 8], FP32, tag="rxy")
                      nc.sync.dma_start(out=rxy[:nq, :], in_=t_rp[q0 : q0 + nq, :])
                      xc = sp2.tile([128, 128], FP32, tag="xc")
                      yc = sp2.tile([128, 128], FP32, tag="yc")
                      for axv, ci, colo in ((xc, 0, 0), (yc, 1, 1)):
                          nc.vector.tensor_tensor(
                              axv[:nq, :].rearrange("q (l h p) -> q l h p", l=NL, h=NH),
                              _bc(
                                  _bc(
                                      rxy[:nq, :].rearrange("q (l t) -> q l t", t=2)[:, :, colo],
                                      2, NH,
                                  ),
                                  3, NP,
                              ),
                              cv(ci).rearrange("q (l h p) -> q l h p", l=NL, h=NH)[:nq],
                              MUL,
                          )
                          nc.vector.tensor_tensor(
                              axv[:nq, :],
                              axv[:nq, :],
                              off[:nq, :].rearrange("q (s t) -> q s t", t=2)[:, :, colo],
                              ADD,
                          )

                      # floor via magic round + correction, then clamp
                      def floor_clamp(src, tagp, hivk):
                          f = sp2.tile([128, 128], FP32, tag="f" + tagp)
                          nc.vector.tensor_scalar_add(f[:nq, :], src[:nq, :], MAGIC)
                          nc.vector.tensor_scalar_sub(f[:nq, :], f[:nq, :], MAGIC)
                          g = sp2.tile([128, 128], FP32, tag="g" + tagp)
                          nc.vector.tensor_tensor(g[:nq, :], f[:nq, :], src[:nq, :], mybir.AluOpType.is_gt)
                          nc.vector.tensor_tensor(f[:nq, :], f[:nq, :], g[:nq, :], SUB)
                          return f

                      x0f = floor_clamp(xc, "x", 0)
                      y0f = floor_clamp(yc, "y", 1)

                      wx1 = sp2.tile([128, 128], FP32, tag="wx1")
                      wy1 = sp2.tile([128, 128], FP32, tag="wy1")
                      nc.vector.tensor_tensor(wx1[:nq, :], xc[:nq, :], x0f[:nq, :], SUB)
                      nc.vector.tensor_tensor(wy1[:nq, :], yc[:nq, :], y0f[:nq, :], SUB)

                      def corner_w(wf1, f, axis, k1, k2):
                          a0 = sp2.tile([128, 128], FP32, tag="a0" + axis)
                          a1 = sp2.tile([128, 128], FP32, tag="a1" + axis)
                          m = sp2.tile([128, 128], FP32, tag="m" + axis)
                          nc.vector.tensor_scalar(a0[:nq, :], wf1[:nq, :], -1.0, 1.0, MUL, ADD)
                          nc.vector.tensor_scalar(m[:nq, :], f[:nq, :], 0.0, None, mybir.AluOpType.is_ge)
                          nc.vector.tensor_tensor(a0[:nq, :], a0[:nq, :], m[:nq, :], MUL)
                          nc.vector.tensor_scalar(m[:nq, :], f[:nq, :], -1.0, None, mybir.AluOpType.is_ge)
                          nc.vector.tensor_tensor(a1[:nq, :], wf1[:nq, :], m[:nq, :], MUL)
                          nc.vector.tensor_tensor(m[:nq, :], f[:nq, :], cv(k1)[:nq], mybir.AluOpType.is_le)
                          nc.vector.tensor_tensor(a0[:nq, :], a0[:nq, :], m[:nq, :], MUL)
                          nc.vector.tensor_tensor(m[:nq, :], f[:nq, :], cv(k2)[:nq], mybir.AluOpType.is_le)
                          nc.vector.tensor_tensor(a1[:nq, :], a1[:nq, :], m[:nq, :], MUL)
                          return a0, a1

                      ax0, ax1 = corner_w(wx1, x0f, "x", 2, 3)
                      ay0, ay1 = corner_w(wy1, y0f, "y", 4, 5)
                      nc.vector.tensor_tensor(ay0[:nq, :], ay0[:nq, :], attn[:nq, :], MUL)
                      nc.vector.tensor_tensor(ay1[:nq, :], ay1[:nq, :], attn[:nq, :], MUL)

                      # corner weights, pair-table corner order (dx, dy),
                      # duplicated in adjacent pairs so combine multiplies
                      # keep a packed innermost dim (DVE 2x mode)
                      cw16 = ipool.tile([128, 128, 4, 2], FP16, tag="cw16")
                      for dx, ax in ((0, ax0), (1, ax1)):
                          for dy, ay in ((0, ay0), (1, ay1)):
                              nc.vector.tensor_tensor(
                                  cw16[:nq, :, 2 * dx + dy, 0], ax[:nq, :], ay[:nq, :], MUL
                              )
                      nc.vector.tensor_copy(
                          cw16[:nq, :, :, 1], cw16[:nq, :, :, 0]
                      )

                      # chunk index: u = bconst + y0*W + x0 (group-relative
                      # units);  c = (u>>1) + (u&1)*CA, clamped.
                      uf = sp2.tile([128, 128], FP32, tag="uf")
                      nc.vector.tensor_tensor(uf[:nq, :], y0f[:nq, :], cv(0)[:nq], MUL)
                      nc.vector.tensor_tensor(uf[:nq, :], uf[:nq, :], x0f[:nq, :], ADD)
                      nc.vector.tensor_tensor(uf[:nq, :], uf[:nq, :], cv(6)[:nq], ADD)
                      c32 = bigp.tile([128, 128], FP32, tag="c32")
                      if nq < 128:
                          nc.vector.memset(c32[:, :], 0.0)
                      th = sp2.tile([128, 128], FP32, tag="th")
                      nc.vector.tensor_scalar_mul(th[:nq, :], uf[:nq, :], 0.5)
                      f2 = sp2.tile([128, 128], FP32, tag="f2")
                      nc.vector.tensor_scalar_add(f2[:nq, :], th[:nq, :], MAGIC)
                      nc.vector.tensor_scalar_sub(f2[:nq, :], f2[:nq, :], MAGIC)
                      gg = sp2.tile([128, 128], FP32, tag="gg")
                      nc.vector.tensor_tensor(gg[:nq, :], f2[:nq, :], th[:nq, :], mybir.AluOpType.is_gt)
                      nc.vector.tensor_tensor(f2[:nq, :], f2[:nq, :], gg[:nq, :], SUB)
                      # th-f2 in {0, 0.5};  c = uh + (th-f2)*CA2
                      nc.vector.tensor_tensor(th[:nq, :], th[:nq, :], f2[:nq, :], SUB)
                      nc.vector.tensor_tensor(th[:nq, :], th[:nq, :], cv(7)[:nq], MUL)
                      nc.vector.tensor_tensor(c32[:nq, :], f2[:nq, :], th[:nq, :], ADD)
                      nc.vector.tensor_scalar(c32[:nq, :], c32[:nq, :], 0.0, None, MAXOP)
                      nc.vector.tensor_tensor(c32[:nq, :], c32[:nq, :], cv(8)[:nq], MINOP)

                      c32_t[qg] = c32
                      cw_t[qg] = cw16

                  # wrap into dma_gather int16 index layout:
                  # idxw[p, s*8+qh] = c32[16*qh + p%16, s], all 8
                  # partition-groups identical.
                  def emit_A2(qg):
                      c32 = c32_t[qg]
                      idxw = ipool.tile([128, 1024], INT16, tag="idxw")
                      idxv = idxw[:, :].rearrange("p (s e) -> p s e", e=8)
                      for qh in range(8):
                          m, v = qh // 4, qh % 4
                          wp = ppool.tile([128, 128], FP32, tag="tpsum")
                          nc.tensor.matmul(
                              wp[:, :],
                              rep4[64 * m : 64 * (m + 1), v, :],
                              c32[64 * m : 64 * (m + 1), :],
                              start=True, stop=True,
                          )
                          nc.vector.tensor_copy(idxv[:, :, qh], wp[:, :])
                      idx_t[qg] = idxw

                  # ===== gather + partial-combine helpers =====
                  # SWDGE descriptor ring caps one dma_gather's num_idxs;
                  # split larger slot ranges into MAXG-slot chunks.
                  MAXG = 8

                  def gath(dst_view, li, g, idxw, s0, ns):
                      in_ap = tabg[li][g][:, :].rearrange("(c a) b -> c (a b)", a=2)
                      for c0 in range(0, ns, MAXG):
                          cn = min(MAXG, ns - c0)
                          nc.gpsimd.dma_gather(
                              dst_view[:, c0 : c0 + cn, :],
                              in_ap,
                              idxw[:, (s0 + c0) * 8 : (s0 + c0 + cn) * 8],
                              cn * 128,
                              cn * 128,
                              128,
                          )

                  def tree_hpc(ps, nq, out_hb):
                      """In-place sum over (p, c) of ps[:, :4096] (h p c ch
                      layout), final level writes out_hb [q, (h 32)]."""
                      cur = 4096
                      for _ in range(4):
                          nxt = cur // 2
                          v = ps[:nq, 0:cur].rearrange("q (h a b) -> q h a b", h=NH, a=2)
                          if nxt == 256:
                              dst = out_hb
                          else:
                              dst = ps[:nq, 0:nxt].rearrange("q (h b) -> q h b", h=NH)
                          nc.vector.tensor_tensor(dst, v[:, :, 0, :], v[:, :, 1, :], ADD)
                          cur = nxt

                  # ===== Phase 1 + loops B, C ====================================
                  with (
                    tc.tile_pool(name=f"vsb{rep}", bufs=1) as vpool,
                    tc.tile_pool(name=f"p1w{rep}", bufs=2) as wp1,
                    tc.tile_pool(name=f"p1d{rep}", bufs=1, space="DRAM") as dp1,
                  ):
                    v_all = vpool.tile([128, NT_TOT, D], FP16)
                    xf16 = dp1.tile([XROWS, D], FP16)

                    def p1_bands(li, b_lo, b_hi):
                        H, W = SPATIAL[li]
                        p0 = XB[li]
                        nt = LVL_NT[li]
                        t0 = LVL_T0[li]
                        for band in range(b_lo, b_hi):
                            xTb = wp1.tile([128, 2, 1024], FP16, tag="xTb")
                            for j in range(2):
                                nc.sync.dma_start_transpose(
                                    out=xTb[:, j, :],
                                    in_=xf16[p0 + band * 1024 : p0 + (band + 1) * 1024, j * 128 : (j + 1) * 128],
                                )
                            for tloc in range(8):
                                t = band * 8 + tloc
                                if t >= nt:
                                    break
                                ti = t0 + t
                                vp = ppool.tile([128, D], FP32, tag="vpp")
                                nc.tensor.matmul(vp[:], xTb[:, 0, tloc * 128 : (tloc + 1) * 128], wv16[:, 0, :], start=True, stop=False)
                                nc.tensor.matmul(vp[:], xTb[:, 1, tloc * 128 : (tloc + 1) * 128], wv16[:, 1, :], start=False, stop=False)
                                nc.tensor.matmul(vp[:], ones16[:, :], bv16[:], start=False, stop=True)
                                nc.scalar.copy(out=v_all[:, ti, :], in_=vp[:])

                    def p1_write_head(li, h, t_lo, t_hi):
                        H, W = SPATIAL[li]
                        nt = LVL_NT[li]
                        t0 = LVL_T0[li]
                        tab = tabg[li][_grp_of_head(li, h)]
                        b = _head_base(li, h)
                        src = v_all[:, t0 + t_lo : t0 + t_hi, h * DH : (h + 1) * DH]
                        dstA = tab[
                            b + W + t_lo * 128 : b + W + t_hi * 128, 0:32
                        ].rearrange("(t p) c -> p t c", p=128)
                        dstB = tab[
                            b + t_lo * 128 : b + t_hi * 128, 32:64
                        ].rearrange("(t p) c -> p t c", p=128)
                        nc.sync.dma_start(out=dstA, in_=src)
                        nc.scalar.dma_start(out=dstB, in_=src)

                    def p1_bcopy(li, g, part, nparts):
                        # parity copy of the g-th group's A region, piecewise:
                        # part k of nparts (each piece only needs the A units
                        # it reads, so pieces can fire as heads finish)
                        half = GRP_UNITS[li] // 2
                        n = half - 1
                        lo = part * n // nparts
                        hi = (part + 1) * n // nparts
                        nc.scalar.dma_start(
                            out=tabg[li][g][half + lo : half + hi, :],
                            in_=tabg[li][g][1 + lo : 1 + hi, :],
                        )

                    def p1_writes(li):
                        nt = LVL_NT[li]
                        for g in range(NGRP[li]):
                            for h in ([2 * g, 2 * g + 1] if li == 0 else range(NH)):
                                p1_write_head(li, h, 0, nt)
                            p1_bcopy(li, g, 0, 1)

                    def phase1_level(li):
                        p1_bands(li, 0, LVL_NB[li])
                        p1_writes(li)

                    if "phase1" not in ablate:
                        # per-level fp16 copy of X in DRAM (cast during DMA),
                        # all four casts up-front so nothing queues behind
                        # the gathers on the Pool engine.
                        for li in (3, 2, 1, 0):
                            H, W = SPATIAL[li]
                            npos = H * W
                            p0 = XB[li]
                            nc.gpsimd.dma_start(
                                out=xf16[p0 : p0 + npos, :],
                                in_=t_x[LVL_START[li] : LVL_START[li] + npos, :],
                            )
                            r = p0 + npos
                            while r < p0 + LVL_NB[li] * 1024:
                                k = min(128, p0 + LVL_NB[li] * 1024 - r)
                                nc.gpsimd.dma_start(
                                    out=xf16[r : r + k, :].rearrange("r (a c) -> r a c", a=4),
                                    in_=_bc(zt[:k, 0, :], 1, 4),
                                )
                                r += k
                    # ---- loop B body: small-level gathers + partial combine
                    def emit_B(qg):
                        q0 = qg * 128
                        nq = min(128, Q - q0)
                        idxw, cw16 = idx_t[qg], cw_t[qg]
                        ogp = ipool.tile([128, D], FP16, tag="ogp")
                        if "gather" in ablate or "combine" in ablate:
                            nc.vector.memset(ogp[:, :], 0.0)
                        if nq < 128 and "combine" not in ablate:
                            nc.vector.memset(ogp[:, :], 0.0)
                        for li in (3, 2, 1):
                            ps = bigp.tile([128, 4096], FP16, tag="psml")
                            if "gather" in ablate:
                                nc.vector.memset(ps[:, :], 0.0)
                            else:
                                gath(
                                    ps[:, :].rearrange("q (s e) -> q s e", e=128),
                                    li, 0, idxw, 32 * li, 32,
                                )
                            if "combine" in ablate:
                                continue
                            nc.vector.tensor_tensor(
                                ps[:nq, :].rearrange("q (s c k two) -> q s c k two", s=32, c=4, two=2),
                                ps[:nq, :].rearrange("q (s c k two) -> q s c k two", s=32, c=4, two=2),
                                _bc(cw16[:nq, 32 * li : 32 * (li + 1), :, :], 3, DH // 2),
                                MUL,
                            )
                            tree_hpc(ps, nq, ps[:nq, 0:256].rearrange("q (h b) -> q h b", h=NH))
                            if li == 3:
                                nc.vector.tensor_copy(ogp[:nq, :], ps[:nq, 0:256])
                            else:
                                nc.vector.tensor_tensor(
                                    ogp[:nq, :], ogp[:nq, :], ps[:nq, 0:256], ADD
                                )
                        ogp_t[qg] = ogp

                    # ---- loop C body: lvl0 gathers + final combine + output
                    og_t = {}

                    def emit_C_half(qg, half):
                        q0 = qg * 128
                        nq = min(128, Q - q0)
                        idxw, cw16 = idx_t[qg], cw_t[qg]
                        og = ogp_t[qg]
                        p0t = bigp.tile([128, 2048], FP16, tag="p0h")
                        if "gather" in ablate:
                            nc.vector.memset(p0t[:, :], 0.0)
                        else:
                            for gl in range(2):
                                g = 2 * half + gl
                                gath(
                                    p0t[:, gl * 1024 : (gl + 1) * 1024].rearrange(
                                        "q (s e) -> q s e", e=128
                                    ),
                                    0, g, idxw, 8 * g, 8,
                                )
                        if "combine" not in ablate:
                            nc.vector.tensor_tensor(
                                p0t[:nq, :].rearrange("q (s c k two) -> q s c k two", s=16, c=4, two=2),
                                p0t[:nq, :].rearrange("q (s c k two) -> q s c k two", s=16, c=4, two=2),
                                _bc(cw16[:nq, 16 * half : 16 * (half + 1), :, :], 3, DH // 2),
                                MUL,
                            )
                            # sum (p, c) within the 4 heads of this half,
                            # accumulate into the ogp partial
                            cur = 2048
                            while cur > 128:
                                nxt = cur // 2
                                v = p0t[:nq, 0:cur].rearrange("q (h a b) -> q h a b", h=4, a=2)
                                dst = p0t[:nq, 0:nxt].rearrange("q (h b) -> q h b", h=4)
                                nc.vector.tensor_tensor(dst, v[:, :, 0, :], v[:, :, 1, :], ADD)
                                cur = nxt
                            sl = og[:nq, half * 128 : (half + 1) * 128]
                            nc.vector.tensor_tensor(sl, sl, p0t[:nq, 0:128], ADD)
                        if half == 0:
                            return

                        # ---- output projection ----
                        ogT = wp2.tile([128, 2, 128], FP16, tag="ogT")
                        for j in range(2):
                            tp2 = ppool.tile([128, 128], FP16, tag="tpsum16")
                            nc.tensor.transpose(
                                tp2[:, :], og[:, j * 128 : (j + 1) * 128], ident16[:, :]
                            )
                            nc.scalar.copy(out=ogT[:, j, :], in_=tp2[:, :])
                        outp = ppool.tile([128, D], FP32, tag="mmout")
                        nc.tensor.matmul(outp[:nq, :], ogT[:, 0, :nq], wo16[:, 0, :], start=True, stop=False)
                        nc.tensor.matmul(outp[:nq, :], ogT[:, 1, :nq], wo16[:, 1, :], start=False, stop=False)
                        nc.tensor.matmul(outp[:nq, :], ones16[:, :nq], bo16[:], start=False, stop=True)
                        ofin = wp2.tile([128, D], FP32, tag="ofin")
                        nc.scalar.copy(out=ofin[:nq, :], in_=outp[:nq, :])
                        nc.sync.dma_start(out=t_out[q0 : q0 + nq, :], in_=ofin[:nq, :])

                    # ===== interleaved emission plan =====
                    p1 = "phase1" not in ablate
                    emit_A(0)
                    emit_A(1)
                    if p1 and rep == 0:
                        for li in (3, 2, 1, 0):
                            zfill_level(li)
                    emit_A2(0)
                    emit_A(2)
                    if p1: phase1_level(3)
                    emit_A2(1)
                    emit_A(3)
                    if p1: phase1_level(2)
                    emit_A2(2)
                    emit_A(4)
                    if p1: phase1_level(1)
                    emit_A2(3)
                    emit_B(0)
                    emit_B(1)
                    emit_A(5)
                    if p1: p1_bands(0, 0, 3)
                    emit_A2(4)
                    emit_B(2)
                    emit_A(6)
                    if p1: p1_bands(0, 3, 6)
                    emit_A2(5)
                    emit_B(3)
                    emit_A(7)
                    if p1: p1_bands(0, 6, 9)
                    emit_A2(6)
                    emit_B(4)
                    if p1: p1_bands(0, 9, 12)
                    emit_A2(7)
                    emit_B(5)
                    if p1: p1_bands(0, 12, LVL_NB[0])
                    emit_B(6)
                    emit_B(7)
                    if p1:
                        nt0 = LVL_NT[0]
                        for g in range(4):
                            for h in (2 * g, 2 * g + 1):
                                p1_write_head(0, h, 0, nt0 // 2)
                        for g in range(4):
                            p1_write_head(0, 2 * g, nt0 // 2, nt0)
                            p1_bcopy(0, g, 0, 2)
                            p1_write_head(0, 2 * g + 1, nt0 // 2, nt0)
                            p1_bcopy(0, g, 1, 2)
                    for qg in range(NQG):
                        emit_C_half(qg, 0)
                    for qg in range(NQG):
                        emit_C_half(qg, 1)

    nc.compile()
    return nc


_NC_CACHE = None


def kernel(**inputs) -> np.ndarray:
    global _NC_CACHE
    if _NC_CACHE is None:
        _NC_CACHE = build()
    nc = _NC_CACHE
    cvec, rep16 = _np_consts()
    w_off2, b_off2, w_attn2, b_attn2 = _np_permuted_weights(
        np.asarray(inputs["W_off"], np.float32),
        np.asarray(inputs["b_off"], np.float32),
        np.asarray(inputs["W_attn"], np.float32),
        np.asarray(inputs["b_attn"], np.float32),
    )
    cvec2 = np.ascontiguousarray(cvec.reshape(128, 9 * 128))
    in_maps = []
    for b in range(BS):
        in_maps.append(
            {
                "query": np.ascontiguousarray(inputs["query"][b], np.float32),
                "reference_points": np.ascontiguousarray(
                    inputs["reference_points"][b], np.float32
                ).reshape(Q, NL * 2),
                "input_flatten": np.ascontiguousarray(inputs["input_flatten"][b], np.float32),
                "W_off": w_off2,
                "b_off": b_off2,
                "W_attn": w_attn2,
                "b_attn": b_attn2,
                "W_v": np.ascontiguousarray(inputs["W_v"], np.float32),
                "b_v": np.ascontiguousarray(inputs["b_v"], np.float32),
                "W_o": np.ascontiguousarray(inputs["W_o"], np.float32),
                "b_o": np.ascontiguousarray(inputs["b_o"], np.float32),
                "cvec": cvec2,
                "rep16": rep16,
            }
        )
    res = run_bass_kernel_spmd(nc, in_maps, core_ids=list(range(BS)))
    return np.stack([res.results[b]["out"] for b in range(BS)], axis=0)
